# revision 14
# baseline (speedup 1.0000x reference)
"""DecompGridv3 embedding lookup on 8 Trainium2 NeuronCores — v2.

Data-parallel over B=1M query points (128K/core, one point per
(partition, j-col)).  Host pre-bakes gather-friendly fp16 tables with the
interpolation corners *interleaved per feature* (row[f*C + c]) so that on
device:
  - ONE batched indirect DMA per (tile, table) gathers all corners for
    Jt*128 points (descriptors: grid 512B, planes 256B, line 128B),
  - the corner-weight multiply is a step-1 fp16 tensor_tensor (DVE 2x mode)
    against a [p, j, (f:step0), (c:step1)] weight view,
  - the corner reduction is a contiguous-halves add tree.

Tables:
  tabg [128^3, 32*8]  fp16: z8 brick — all 8 trilinear corners, c=(dz,dy,dx)
  tabp [3*384^2, 32*4] fp16: 2x2 patch quad brick, c=(dx,dy), planes 01/02/12
  tabl [256, 32*2]    fp16: line pair, c=(i, i+1)

Engines: Pool (gpsimd) runs the SWDGE gathers + small 1x ops, DVE runs the
big fp16 2x multiplies/adds, ACT does scale/bias coordinate prep, HWDGE
(sync) does the streaming x-in / out DMAs.
"""

import contextlib
import numpy as np

import concourse.bacc as bacc
import concourse.tile as tile
import concourse.mybir as mybir
from concourse.bass import AP, IndirectOffsetOnAxis
from concourse.bass_utils import run_bass_kernel_spmd

F32 = mybir.dt.float32
F16 = mybir.dt.float16
I32 = mybir.dt.int32
ALU = mybir.AluOpType
ACTF = mybir.ActivationFunctionType

NF = 32
D3 = 128
P2 = 384
L1 = 256
B = 1 << 20
NCORES = 8

JTOT = (B // NCORES) // 128       # 1024 j-cols per core
CHUNK = 64                        # j-cols per coord/weight chunk
TILE = 16                         # j-cols per gather tile
REPEAT = 1


def _ins0(ap: AP, pos: int, count: int) -> AP:
    """Insert a broadcast (step-0) dim at `pos` of ap's dim list."""
    dims = [list(d) for d in ap.ap]
    dims.insert(pos, [0, count])
    return AP(ap.tensor, ap.offset, dims)


def _apv(ap: AP, extra_off: int, dims) -> AP:
    """Manual AP view: keep ap's partition dim, replace free dims."""
    return AP(ap.tensor, ap.offset + extra_off,
              [list(ap.ap[0])] + [list(d) for d in dims])


def build_bass(d3=D3, p2=P2, l1=L1, jtot=JTOT, chunk=CHUNK, tsz=TILE,
               repeat=REPEAT, ncores=NCORES):
    grows = d3 * d3 * d3
    prows = 3 * p2 * p2
    nchunk = jtot // chunk
    ntile = chunk // tsz
    J = chunk
    Jt = tsz

    nc = bacc.Bacc("TRN2", target_bir_lowering=False, debug=False,
                   num_devices=ncores)
    xin = nc.dram_tensor("xin", [128, jtot * 4], F32, kind="ExternalInput")
    tabg = nc.dram_tensor("tabg", [grows, NF * 8], F16, kind="ExternalInput")
    tabp = nc.dram_tensor("tabp", [prows, NF * 4], F16, kind="ExternalInput")
    # line table padded to 128 cols so dma_gather elem_size_bytes % 256 == 0
    tabl = nc.dram_tensor("tabl", [l1, NF * 4], F16, kind="ExternalInput")
    xl16 = nc.dram_tensor("xl16", [16, jtot * 8], F32, kind="ExternalInput")
    out = nc.dram_tensor("out", [128, jtot * NF], F32, kind="ExternalOutput")
    I16 = mybir.dt.int16

    with tile.TileContext(nc) as tc:
        with contextlib.ExitStack() as ctx:
            cp = ctx.enter_context(tc.tile_pool(name="cp", bufs=2))
            gp = ctx.enter_context(tc.tile_pool(name="gp", bufs=3))
            rp = ctx.enter_context(tc.tile_pool(name="rp", bufs=2))
            op = ctx.enter_context(tc.tile_pool(name="op", bufs=3))

            rep_ctx = (tc.For_i(0, repeat, 1) if repeat > 1
                       else contextlib.nullcontext())
            with rep_ctx:
              for c in range(nchunk):
                j0 = c * J
                # ---------------- chunk-level: coords, offsets, weights
                xs = cp.tile([128, J, 4], F32, tag="xs")
                nc.sync.dma_start(
                    xs[:], xin.ap()[:, j0 * 4:(j0 + J) * 4]
                           .rearrange("p (j c) -> p j c", c=4))

                # tcoord[p, k, j] = (x_k + 1) * 0.5   (k = 0..2)
                tco = cp.tile([128, 3, J], F32, tag="tco")
                nc.scalar.activation(
                    tco[:], xs[:, :, 0:3].rearrange("p j k -> p k j"),
                    ACTF.Copy, bias=0.5, scale=0.5)
                # fv[p, s, j]: s=0..2 grid fx/fy/fz; 3..5 plane; 6 line
                fv = cp.tile([128, 7, J], F32, tag="fv")
                nc.scalar.activation(fv[:, 0:3, :], tco[:], ACTF.Copy,
                                     bias=0.0, scale=float(d3 - 1))
                nc.scalar.activation(fv[:, 3:6, :], tco[:], ACTF.Copy,
                                     bias=0.0, scale=float(p2 - 1))
                nc.scalar.activation(
                    fv[:, 6:7, :],
                    xs[:, :, 3:4].rearrange("p j k -> p k j"),
                    ACTF.Copy, bias=0.0, scale=float(l1))

                # floor + frac (round trick), all 7 streams at once
                ri = cp.tile([128, 7, J], I32, tag="ri")
                nc.vector.tensor_copy(ri[:], fv[:])
                rf = cp.tile([128, 7, J], F32, tag="rf")
                nc.vector.tensor_copy(rf[:], ri[:])
                m = cp.tile([128, 7, J], F32, tag="m")
                nc.vector.tensor_tensor(out=m[:], in0=rf[:], in1=fv[:],
                                        op=ALU.is_gt)
                fl = cp.tile([128, 7, J], F32, tag="fl")
                nc.vector.tensor_sub(fl[:], rf[:], m[:])
                # pair table P7[p, s, j, 0] = 1-w, [.., 1] = w
                P7 = cp.tile([128, 7, J, 2], F32, tag="P7")
                nc.vector.tensor_tensor(
                    out=P7[:, :, :, 1:2].rearrange("p s j o -> p s (j o)"),
                    in0=fv[:], in1=fl[:], op=ALU.subtract)
                nc.scalar.activation(
                    P7[:, :, :, 0:1].rearrange("p s j o -> p s (j o)"),
                    P7[:, :, :, 1:2].rearrange("p s j o -> p s (j o)"),
                    ACTF.Copy, bias=1.0, scale=-1.0)

                def fls(s):
                    return fl[:, s:s + 1, :].rearrange("p s j -> p (s j)")

                # grid offset rows: z*d3^2 + y*d3 + x
                ga = cp.tile([128, J], F32, tag="ga")
                nc.vector.scalar_tensor_tensor(
                    out=ga[:], in0=fls(1), scalar=float(d3), in1=fls(0),
                    op0=ALU.mult, op1=ALU.add)
                gb = cp.tile([128, J], F32, tag="gb")
                nc.vector.scalar_tensor_tensor(
                    out=gb[:], in0=fls(2), scalar=float(d3 * d3), in1=ga[:],
                    op0=ALU.mult, op1=ALU.add)
                offg = cp.tile([128, J], I32, tag="offg")
                nc.vector.tensor_copy(offg[:], gb[:])

                # plane offsets (interleaved [p, j, pl]):
                # row = pl*p2^2 + y*p2 + x ; (y, x) per plane:
                #   p01: (s4, s3)  p02: (s5, s3)  p12: (s5, s4)
                pof = cp.tile([128, J, 3], F32, tag="pof")
                offp = cp.tile([128, J, 3], I32, tag="offp")
                for pl, (sy, sx) in enumerate(((4, 3), (5, 3), (5, 4))):
                    dst = pof[:, :, pl:pl + 1].rearrange("p j o -> p (j o)")
                    nc.vector.scalar_tensor_tensor(
                        out=dst, in0=fls(sy), scalar=float(p2), in1=fls(sx),
                        op0=ALU.mult, op1=ALU.add)
                    nc.vector.tensor_scalar(
                        out=offp[:, :, pl:pl + 1].rearrange("p j o -> p (j o)"),
                        in0=dst, scalar1=float(pl * p2 * p2), scalar2=None,
                        op0=ALU.add)
                # ---- weights: outer products of pair slices (Pool)
                def pair(s):
                    return P7[:, s:s + 1, :, :].rearrange("p s j o -> p (s j) o")

                zy = cp.tile([128, J, 2, 2], F32, tag="zy")
                nc.vector.tensor_mul(zy[:], pair(2).to_broadcast([128, J, 2, 2]),
                                     _ins0(pair(1), 2, 2))
                w8t = cp.tile([128, J, 8], F16, tag="w8t")
                nc.vector.tensor_mul(
                    w8t[:].rearrange("p j (a b) -> p j a b", b=2),
                    zy[:].rearrange("p j a b -> p j (a b)")
                         .to_broadcast([128, J, 4, 2]),
                    _ins0(pair(0), 2, 4))
                wpt = cp.tile([128, J, 3, 4], F16, tag="wpt")
                for pl, (sx, sy) in enumerate(((3, 4), (3, 5), (4, 5))):
                    nc.vector.tensor_mul(
                        wpt[:, :, pl:pl + 1, :]
                            .rearrange("p j o (a b) -> p j (o a) b", b=2),
                        pair(sx).to_broadcast([128, J, 2, 2]),
                        _ins0(pair(sy), 2, 2))
                wlt = cp.tile([128, J, 2], F16, tag="wlt")
                nc.vector.tensor_copy(wlt[:], pair(6))

                # ---- line: int16 idx on 16 partitions (host xl16 layout),
                # replicate, one batched dma_gather for the chunk
                li_f = cp.tile([16, J * 8], F32, tag="li_f")
                nc.sync.dma_start(li_f[:],
                                  xl16.ap()[:, c * J * 8:(c + 1) * J * 8])
                lfv = cp.tile([16, J * 8], F32, tag="lfv")
                nc.vector.tensor_scalar(out=lfv[:], in0=li_f[:],
                                        scalar1=float(l1), scalar2=None,
                                        op0=ALU.mult)
                lri = cp.tile([16, J * 8], I32, tag="lri")
                nc.vector.tensor_copy(lri[:], lfv[:])
                lrf = cp.tile([16, J * 8], F32, tag="lrf")
                nc.vector.tensor_copy(lrf[:], lri[:])
                lm = cp.tile([16, J * 8], F32, tag="lm")
                nc.vector.tensor_tensor(out=lm[:], in0=lrf[:], in1=lfv[:],
                                        op=ALU.is_gt)
                lfl = cp.tile([16, J * 8], F32, tag="lfl")
                nc.vector.tensor_sub(lfl[:], lrf[:], lm[:])
                lidx = cp.tile([128, J * 8], I16, tag="lidx")
                nc.vector.tensor_copy(lidx[0:16, :], lfl[:])
                for grp in range(1, 8):
                    nc.sync.dma_start(lidx[16 * grp:16 * (grp + 1), :],
                                      lidx[0:16, :])
                ld = cp.tile([128, J, NF * 4], F16, tag="ld")
                nc.gpsimd.dma_gather(
                    out_ap=ld[:], in_ap=tabl.ap(), idxs_ap=lidx[:],
                    num_idxs=J * 128, num_idxs_reg=J * 128,
                    elem_size=NF * 4, single_packet=False)

                # ---------------- tile-level: gathers + interp
                for t in range(ntile):
                    ts0 = t * Jt
                    gg = gp.tile([128, Jt, NF * 8], F16, tag="gg")
                    pg = gp.tile([128, Jt, 3, NF * 4], F16, tag="pg")
                    for u in range(Jt):
                        j = ts0 + u
                        nc.gpsimd.indirect_dma_start(
                            out=gg[:, u:u + 1, :]
                                .rearrange("p o e -> p (o e)"),
                            out_offset=None, in_=tabg.ap(),
                            in_offset=IndirectOffsetOnAxis(
                                ap=offg[:, j:j + 1], axis=0))
                        for pl in range(3):
                            nc.gpsimd.indirect_dma_start(
                                out=pg[:, u:u + 1, pl:pl + 1, :]
                                    .rearrange("p o l e -> p (o l e)"),
                                out_offset=None, in_=tabp.ap(),
                                in_offset=IndirectOffsetOnAxis(
                                    ap=offp[:, j:j + 1, pl:pl + 1]
                                        .rearrange("p o l -> p (o l)"),
                                    axis=0))

                    # weight multiply (in-place, fp16 2x)
                    gv = gg[:].rearrange("p j (f c) -> p j f c", c=8)
                    nc.vector.tensor_mul(
                        gv, gv, _ins0(w8t[:, ts0:ts0 + Jt, :], 2, NF))
                    pv = pg[:].rearrange("p j l (f c) -> p j l f c", c=4)
                    nc.vector.tensor_mul(
                        pv, pv, _ins0(wpt[:, ts0:ts0 + Jt, :, :], 3, NF))
                    ldap = ld[:]
                    lv = _apv(ldap, ts0 * NF * 4,
                              [[NF * 4, Jt], [2, NF], [1, 2]])
                    nc.vector.tensor_mul(
                        lv, lv, _ins0(wlt[:, ts0:ts0 + Jt, :], 2, NF))

                    # corner reduction trees
                    # grid: [Jt, f, 8] -> halves -> [Jt, f, 4] -> ... -> [Jt, f]
                    ggap = gg[:]
                    g4 = rp.tile([128, Jt, NF, 4], F16, tag="g4")
                    nc.vector.tensor_add(
                        g4[:],
                        _apv(ggap, 0, [[NF * 8, Jt], [8, NF], [1, 4]]),
                        _apv(ggap, 4, [[NF * 8, Jt], [8, NF], [1, 4]]))
                    g4ap = g4[:]
                    g2lo = _apv(g4ap, 0, [[NF * 4, Jt], [4, NF], [1, 2]])
                    nc.vector.tensor_add(
                        g2lo, g2lo,
                        _apv(g4ap, 2, [[NF * 4, Jt], [4, NF], [1, 2]]))
                    g1 = rp.tile([128, Jt, NF], F16, tag="g1")
                    nc.vector.tensor_add(
                        g1[:],
                        _apv(g4ap, 0, [[NF * 4, Jt], [4, NF]]),
                        _apv(g4ap, 1, [[NF * 4, Jt], [4, NF]]))
                    # planes: [Jt, 3, f, 4] -> [Jt, 3, f, 2] -> [Jt, 3, f]
                    pgap = pg[:]
                    p2t = rp.tile([128, Jt, 3, NF, 2], F16, tag="p2t")
                    nc.vector.tensor_add(
                        p2t[:],
                        _apv(pgap, 0, [[NF * 12, Jt], [NF * 4, 3], [4, NF], [1, 2]]),
                        _apv(pgap, 2, [[NF * 12, Jt], [NF * 4, 3], [4, NF], [1, 2]]))
                    p2ap = p2t[:]
                    p1 = rp.tile([128, Jt, 3, NF], F16, tag="p1")
                    nc.vector.tensor_add(
                        p1[:],
                        _apv(p2ap, 0, [[NF * 6, Jt], [NF * 2, 3], [2, NF]]),
                        _apv(p2ap, 1, [[NF * 6, Jt], [NF * 2, 3], [2, NF]]))
                    # line: [Jt, f, 2] -> [Jt, f]
                    l1t = rp.tile([128, Jt, NF], F16, tag="l1t")
                    nc.vector.tensor_add(
                        l1t[:],
                        _apv(ldap, ts0 * NF * 4, [[NF * 4, Jt], [2, NF]]),
                        _apv(ldap, ts0 * NF * 4 + 1, [[NF * 4, Jt], [2, NF]]))

                    # final products
                    t1 = rp.tile([128, Jt, NF], F16, tag="t1")
                    nc.vector.tensor_mul(t1[:], g1[:], l1t[:])
                    t2 = rp.tile([128, Jt, NF], F16, tag="t2")
                    nc.vector.tensor_mul(t2[:], p1[:, :, 0, :], p1[:, :, 1, :])
                    t3 = rp.tile([128, Jt, NF], F16, tag="t3")
                    nc.vector.tensor_mul(t3[:], t2[:], p1[:, :, 2, :])
                    ot = op.tile([128, Jt, NF], F32, tag="ot")
                    nc.vector.tensor_mul(ot[:], t3[:], t1[:])
                    nc.sync.dma_start(
                        out.ap()[:, (j0 + ts0) * NF:(j0 + ts0 + Jt) * NF],
                        ot[:].rearrange("p u f -> p (u f)"))

    nc.compile()
    return nc


def _prep_tables(grid3d, plane01, plane02, plane12, line0,
                 d3=D3, p2=P2, l1=L1):
    f = grid3d.shape[0]
    gt = np.ascontiguousarray(
        grid3d.transpose(1, 2, 3, 0)).astype(np.float16)       # (z, y, x, f)
    gpad = np.empty((d3 + 1, d3 + 1, d3 + 1, f), np.float16)
    gpad[:d3, :d3, :d3] = gt
    gpad[d3, :d3, :d3] = gt[d3 - 1]
    gpad[:, d3, :d3] = gpad[:, d3 - 1, :d3]
    gpad[:, :, d3] = gpad[:, :, d3 - 1]
    del gt
    tabg = np.empty((d3, d3, d3, f, 8), np.float16)
    for dz in range(2):
        for dy in range(2):
            for dx in range(2):
                cidx = dz * 4 + dy * 2 + dx
                tabg[..., cidx] = gpad[dz:dz + d3, dy:dy + d3, dx:dx + d3, :]
    del gpad
    tabg = tabg.reshape(d3 ** 3, f * 8)

    def quad(p):
        pt = np.ascontiguousarray(p.transpose(1, 2, 0)).astype(np.float16)
        ppad = np.empty((p2 + 1, p2 + 1, f), np.float16)
        ppad[:p2, :p2] = pt
        ppad[p2, :p2] = pt[p2 - 1]
        ppad[:, p2] = ppad[:, p2 - 1]
        t = np.empty((p2, p2, f, 4), np.float16)
        for dx in range(2):
            for dy in range(2):
                t[..., dx * 2 + dy] = ppad[dy:dy + p2, dx:dx + p2, :]
        return t.reshape(p2 * p2, f * 4)

    tabp = np.concatenate([quad(plane01), quad(plane02), quad(plane12)], 0)

    lt = np.ascontiguousarray(line0.T).astype(np.float16)       # (l1, f)
    tl = np.empty((l1, f, 2), np.float16)
    tl[:, :, 0] = lt
    tl[:-1, :, 1] = lt[1:]
    tl[-1, :, 1] = lt[-1]
    tabl = np.zeros((l1, f * 4), np.float16)
    tabl[:, :f * 2] = tl.reshape(l1, f * 2)
    return tabg, tabp, tabl


_NC_CACHE = {}


def kernel(x, grid3d, plane01, plane02, plane12, line0):
    x = np.asarray(x, np.float32)
    tabg, tabp, tabl = _prep_tables(
        np.asarray(grid3d, np.float32), np.asarray(plane01, np.float32),
        np.asarray(plane02, np.float32), np.asarray(plane12, np.float32),
        np.asarray(line0, np.float32))
    if "nc" not in _NC_CACHE:
        _NC_CACHE["nc"] = build_bass()
    nc = _NC_CACHE["nc"]

    bcore = B // NCORES
    in_maps = []
    for cix in range(NCORES):
        xc = x[cix * bcore:(cix + 1) * bcore]
        in_maps.append(core_inmap(xc, tabg, tabp, tabl))
    res = run_bass_kernel_spmd(nc, in_maps, core_ids=list(range(NCORES)))
    outs = [r["out"].reshape(bcore, NF) for r in res.results]
    return np.concatenate(outs, axis=0)


def core_inmap(xc, tabg, tabp, tabl, jtot=JTOT, chunk=CHUNK):
    """Per-core inputs from this core's (bcore, 4) slice of x."""
    nchunk = jtot // chunk
    xinc = np.ascontiguousarray(xc.reshape(128, jtot * 4))
    # xl16[q, c*8J + j*8 + k] = x3 of point (p=16k+q, j = c*J + j)
    x3 = xc[:, 3].reshape(128, jtot)
    x3g = x3.reshape(8, 16, nchunk, chunk)          # [k, q, c, j]
    xl16c = np.ascontiguousarray(
        x3g.transpose(1, 2, 3, 0).reshape(16, jtot * 8))
    return {"xin": xinc, "tabg": tabg, "tabp": tabp, "tabl": tabl,
            "xl16": xl16c}
